# revision 23
# baseline (speedup 1.0000x reference)
"""ALiBi multi-head attention on 8 TRN2 NeuronCores — collective-free.

Sharding: data-parallel over batch (B=2 -> 2 groups of 4 cores), tensor-parallel
over the 16 heads (4 heads per core, Megatron-style column shards of Wq/Wk/Wv).
Each core applies its own ROW shard of Wo to its local normalized attention
outputs and returns a partial [S, 1024] sum; the host adds the four partials
per batch group.  No collectives at all: no CC-queue serialization, no
gather-back DMAs blocking the Sync queue, no cross-core skew on the critical
path, and the Wo matmuls pipeline per q-unit instead of waiting for gathers.

Head assignment is slot-ranked: core group-position j gets heads
{15-j, 11-j, 7-j, 3-j} (slot s holds head BINS[j][s]).  Slots are processed in
parity PAIRS (3,2) then (1,0): the two score matmuls of a pair have K=64 and
live on complementary 64-partition halves (po=64*(s%2)), so the PE runs them
CONCURRENTLY in disjoint row-groups (tile_position auto-derived from
base_partition) — near-2x score throughput.  Each pair writes one [128,1024]
PSUM tile and gets a single pair-batched exp on the Scalar engine (the
352-cycle per-ACTIVATE overhead is paid once per two atoms).  Slot-0 atoms
outside slot-1's band are batched two-per-exp the same way.

The ALiBi factor tiles F = exp(-slope*|k-q|) (fp16, Toeplitz: one tile per
tile-diagonal offset) carry each core's actual slopes and double as the band
mask (F underflows to 0 outside a head's band).  Softmax skips the
max-subtraction (scores ~ N(0, 0.45^2), fp32 exp cannot overflow).  V carries
a fused ones-column so the attention-value matmul also emits the softmax
denominators (row 64) for free.

Engine placement: exp on Scalar; F-multiplies + PSUM->SBUF copies on Vector;
the normalize multiplies (SBUF-only) on GpSimd (which has no other work now);
projection copies on Scalar while it is otherwise idle (pre-attention).  K/Q
projections run stationary-weight c-loops (2 q-blocks share each LDWEIGHTS).
"""

import math
import sys

import numpy as np
import ml_dtypes

F8NP = ml_dtypes.float8_e4m3fn

if "/opt/trn_rl_repo" not in sys.path:
    sys.path.insert(0, "/opt/trn_rl_repo")

import concourse.bass as bass  # noqa: E402
import concourse.mybir as mybir  # noqa: E402
import concourse.tile as tile  # noqa: E402
from concourse import bacc  # noqa: E402
from concourse.bass_utils import run_bass_kernel_spmd  # noqa: E402

B, S, D, H, HD = 2, 2048, 1024, 16, 64
NCORES = 8
GROUP = 4          # cores per batch group
NSLOT = 4          # head slots per core
KT = 128           # k (key position) tile size
NKT = S // KT      # 16
NDC = D // 128     # 8 contraction chunks
T_FACTOR = 8.0     # band radius = T_FACTOR / slope (exp(-8) tail ~ 2e-4 bias)

F16 = mybir.dt.float16
F32 = mybir.dt.float32
F8 = mybir.dt.float8e4
W8SCALE = 64.0  # fp8 weight pre-scale (values land in e4m3 normal range)

SLOPES = [2.0 ** (-0.5 * (h + 1)) for h in range(H)]
# BINS[j][s] = head of slot s on cores j and j+4
BINS = [[15 - j, 11 - j, 7 - j, 3 - j] for j in range(GROUP)]
# slot pairs processed (hi, lo): hi at partitions 64:128, lo at 0:64
PAIRS = [(3, 2), (1, 0)]


def _slot_nd():
    """Max |k_tile - q_tile| included per slot (widest head in the slot)."""
    nds = []
    for s in range(NSLOT):
        t = max(
            min(S - 1, int(math.ceil(T_FACTOR / SLOPES[BINS[j][s]])))
            for j in range(GROUP)
        )
        nds.append(min(NKT - 1, (t + KT - 1) // KT))
    return nds


SLOT_ND = _slot_nd()  # [15, 4, 1, 1]
# F tiles must cover offsets (kt - q_tile) for every q_tile in a 4-tile group:
# extend the band by 3 (values there underflow to 0 in fp16 -> free masking).
SLOT_NDE = [min(NKT - 1, nd + (7 if nd >= 8 else 3)) for nd in SLOT_ND]
# per-slot F bases, LOCAL to the fm tensor the slot lives in:
# fm01 holds slots 0,1; fm23 holds slots 2,3.
F_BASE = {}
_acc = 0
for _s in (0, 1):
    F_BASE[_s] = _acc
    _acc += 2 * SLOT_NDE[_s] + 1
NF01 = _acc
_acc = 0
for _s in (2, 3):
    F_BASE[_s] = _acc
    _acc += 2 * SLOT_NDE[_s] + 1
NF23 = _acc


def _f_idx(s, dd):
    """Index of the F tile for slot s, diagonal offset dd = k_tile - q_tile."""
    return F_BASE[s] + SLOT_NDE[s] - dd


def _unit_kts(nd, g):
    return list(range(max(0, 4 * g - nd), min(NKT - 1, 4 * g + 3 + nd) + 1))


def build_graph():
    nc = bacc.Bacc("TRN2", target_bir_lowering=False, debug=False,
                   num_devices=NCORES)
    # xv in k-tile slabs [128, (slab, c, 512)]; xq/xk in q-block slabs.
    xq = nc.dram_tensor("xq", [128, NDC * S], F16, kind="ExternalInput")
    xk = nc.dram_tensor("xk", [128, NDC * S], F16, kind="ExternalInput")
    xv = nc.dram_tensor("xv", [128, NDC * S], F16, kind="ExternalInput")
    wq = nc.dram_tensor("wq", [128, NDC * 256], F16, kind="ExternalInput")
    wk = nc.dram_tensor("wk", [128, NDC * 256], F16, kind="ExternalInput")
    wv = nc.dram_tensor("wv", [128, NDC * 256], F16, kind="ExternalInput")
    # wo: two rhs chunks [128, 1024]: chunk 0 = (slot2, slot3) head rows,
    # chunk 1 = (slot0, slot1) head rows (matching normt partition layout).
    wo = nc.dram_tensor("wo", [128, 2 * 1024], F16, kind="ExternalInput")
    fm01 = nc.dram_tensor("fm01", [128, NF01 * KT], F16, kind="ExternalInput")
    fm23 = nc.dram_tensor("fm23", [128, NF23 * KT], F16, kind="ExternalInput")
    out = nc.dram_tensor("out", [S, 1024], F16, kind="ExternalOutput")

    with tile.TileContext(nc) as tc:
        with (
            tc.tile_pool(name="wpool", bufs=1) as wpool,
            tc.tile_pool(name="persist", bufs=1) as persist,
            tc.tile_pool(name="xpool", bufs=1) as xpool,
            tc.tile_pool(name="dram", bufs=1, space="DRAM") as dram,
            tc.tile_pool(name="pps", bufs=1, space="PSUM") as pps,
            tc.tile_pool(name="psb", bufs=1) as psb,
            tc.tile_pool(name="nsb", bufs=1) as nsb,
        ):
            wv_sb = wpool.tile([128, NDC * 256], F16)
            wq_sb = wpool.tile([128, NDC * 256], F16)
            wk_sb = wpool.tile([128, NDC * 256], F16)
            wo_sb = wpool.tile([128, 2 * 1024], F16)
            f01_sb = wpool.tile([128, NF01 * KT], F16)
            f23_sb = wpool.tile([128, NF23 * KT], F16)

            xv_sl = [xpool.tile([128, NDC * 512], F16, tag="xv", bufs=4,
                                name=f"xv{n}") for n in range(4)]
            xk_sl = [xpool.tile([128, NDC * 512], F16, tag="xkq", bufs=8,
                                name=f"xk{n}") for n in range(4)]
            xq_sl = [xpool.tile([128, NDC * 512], F16, tag="xkq", bufs=8,
                                name=f"xq{n}") for n in range(4)]

            # Q^T/K^T: pair tile t holds slot 2t (partitions 0:64) and slot
            # 2t+1 (partitions 64:128).  V natural layout per k-tile with a
            # ones column per (k_tile, slot) at vx[kt][:, s*65 + 64].
            qt_sb = [persist.tile([128, S], F16, name=f"qt{m}") for m in range(2)]
            kt_sb = [persist.tile([128, S], F16, name=f"kt{m}") for m in range(2)]
            vx_sb = [persist.tile([128, NSLOT * 65], F16, name=f"vx{k}")
                     for k in range(NKT)]
            for k in range(NKT):
                nc.vector.memset(
                    vx_sb[k][:].rearrange("p (s e) -> p s e", s=NSLOT)[:, :, 64:65],
                    1.0,
                )
            # normalized attention outputs: chunk (s2|s3) and (s0|s1),
            # slot s at partitions 64*(s%2).
            normt23 = persist.tile([128, S], F16, name="n23")
            normt01 = persist.tile([128, S], F16, name="n01")
            NTILE = {0: normt01, 1: normt01, 2: normt23, 3: normt23}

            # ---- input DMAs (consumption order) ------------------------------
            def ldx(sl, t, n):
                nc.sync.dma_start(sl[n][:],
                                  t[:, n * NDC * 512:(n + 1) * NDC * 512])

            nc.sync.dma_start(wv_sb[:], wv[:])
            ldx(xv_sl, xv, 0)
            ldx(xv_sl, xv, 1)
            nc.sync.dma_start(wk_sb[:], wk[:])
            nc.sync.dma_start(wq_sb[:], wq[:])
            ldx(xk_sl, xk, 0)
            ldx(xq_sl, xq, 0)
            ldx(xv_sl, xv, 2)
            ldx(xk_sl, xk, 1)
            ldx(xq_sl, xq, 1)
            ldx(xv_sl, xv, 3)
            nc.sync.dma_start(f23_sb[:], fm23[:])
            for n in range(2, 4):
                ldx(xk_sl, xk, n)
                ldx(xq_sl, xq, n)
            nc.sync.dma_start(f01_sb[:], fm01[:])
            nc.sync.dma_start(wo_sb[:], wo[:])

            # preload the exp table set while the PE does projections
            warm = nsb.tile([1, 16], F16, tag="warm")
            nc.scalar.activation(warm[:], vx_sb[0][0:1, 0:16],
                                 mybir.ActivationFunctionType.Exp)

            # ---- projections -------------------------------------------------
            # V (natural layout, fused ones column untouched at col 64).
            # xv slabs are kt-major / c-inner so kt 0 only needs the first
            # quarter of slab 0 (earlier start while the DMA engine ramps).
            for kt in range(NKT):
                ps = pps.tile([128, 1024], F32, tag="sc", bufs=2,
                              name=f"vps{kt}")
                base = (kt % 4) * NDC * 128
                for c in range(NDC):
                    nc.tensor.matmul(
                        ps[:, 0:256],
                        lhsT=xv_sl[kt // 4][:, base + c * 128:
                                            base + c * 128 + 128],
                        rhs=wv_sb[:, c * 256:(c + 1) * 256],
                        start=(c == 0),
                        stop=(c == NDC - 1),
                    )
                nc.scalar.copy(
                    vx_sb[kt][:].rearrange("p (s e) -> p s e", s=NSLOT)[:, :, 0:64],
                    ps[:, 0:256].rearrange("p (s e) -> p s e", e=64),
                )

            # K^T/Q^T: stationary-weight c-loop over two q-blocks at a time
            # (the two MMs of each c share one LDWEIGHTS' worth of re-load).
            def qk_sub(wsb, xsl, dst, mb, nbs, eng):
                ps = pps.tile([128, 1024], F32, tag="aux", bufs=1,
                              name=f"qk{mb}{nbs[0]}")
                for c in range(NDC):
                    for i, nb in enumerate(nbs):
                        nc.tensor.matmul(
                            ps[:, i * 512:(i + 1) * 512],
                            lhsT=wsb[:, c * 256 + mb * 128:
                                     c * 256 + mb * 128 + 128],
                            rhs=xsl[nb][:, c * 512:(c + 1) * 512],
                            start=(c == 0),
                            stop=(c == NDC - 1),
                        )
                for i, nb in enumerate(nbs):
                    eng(dst[:, nb * 512:(nb + 1) * 512],
                        ps[:, i * 512:(i + 1) * 512])

            class _ScalarCopy:
                def tensor_copy(self, dst, src):
                    nc.scalar.copy(dst, src)

            scp = _ScalarCopy()

            def sc_cp(dst, src):
                nc.scalar.copy(dst, src)

            def ve_cp(dst, src):
                nc.vector.tensor_copy(dst, src)

            qk_sub(wk_sb, xk_sl, kt_sb[1], 1, (0, 1), sc_cp)
            qk_sub(wq_sb, xq_sl, qt_sb[1], 1, (0, 1), sc_cp)
            qk_sub(wk_sb, xk_sl, kt_sb[1], 1, (2, 3), sc_cp)
            qk_sub(wk_sb, xk_sl, kt_sb[0], 0, (0, 1), sc_cp)
            qk_sub(wq_sb, xq_sl, qt_sb[1], 1, (2, 3), sc_cp)
            side = [
                lambda: qk_sub(wk_sb, xk_sl, kt_sb[0], 0, (2, 3), ve_cp),
                lambda: qk_sub(wq_sb, xq_sl, qt_sb[0], 0, (0, 1), ve_cp),
                lambda: qk_sub(wq_sb, xq_sl, qt_sb[0], 0, (2, 3), ve_cp),
            ]

            # ---- normalize ---------------------------------------------------
            def keep_pe_warm(lhsT, rhs):
                """Junk matmuls gated on a normalize-chain artifact: keep the
                PE's HAM activity window alive through the final normalize
                latency so the tail Wo matmuls run at full clock."""
                dps = pps.tile([128, 1024], F32, tag="sc", bufs=2, name="dps")
                for r in range(2):
                    nc.tensor.matmul(
                        dps[0:lhsT.free_size(), 0:rhs.shape[-1]],
                        lhsT=lhsT, rhs=rhs, start=True, stop=True,
                    )

            def normalize(s, g, av, mul_eng=None, keepalive=False):
                av_sb = nsb.tile([65, 512], F32, tag="avs", bufs=4)
                nc.vector.tensor_copy(av_sb[:], av[:])
                dsh = nsb.tile([32, 16], F32, tag="dsh", bufs=4)
                nc.sync.dma_start(dsh[:], av_sb[64:65, :])
                rec = nsb.tile([32, 16], F32, tag="rec", bufs=4)
                nc.vector.reciprocal(rec[:], dsh[:])
                rec16 = nsb.tile([32, 16], F16, tag="rec16", bufs=4)
                nc.vector.tensor_copy(rec16[:], rec[:])
                rdr = dram.tile([1, 512], F16, name=f"rdr{s}_{g}",
                                bufs=4, tag="rdr")
                nc.sync.dma_start(rdr[:], rec16[:])
                if keepalive:
                    keep_pe_warm(lhsT=rec16[:], rhs=qt_sb[0][0:32, 0:512])
                bc_sb = nsb.tile([64, 512], F16, tag="bcs", bufs=4)
                nc.sync.dma_start(
                    bc_sb[:],
                    rdr[:].partition_broadcast(64).squeeze(1),
                )
                if keepalive:
                    keep_pe_warm(lhsT=kt_sb[0][0:64, 0:128], rhs=bc_sb[:])
                po = 64 * (s % 2)
                (mul_eng or nc.gpsimd).tensor_mul(
                    NTILE[s][po:po + 64, g * 512:(g + 1) * 512],
                    av_sb[0:64, :],
                    bc_sb[:],
                )

            # ---- Wo (local row shard, per q-tile) ----------------------------
            def emit_wo(m, tag="aux", cp=None):
                po_ = pps.tile([128, 1024], F32, tag=tag, bufs=2 if tag == "sc" else 1,
                               name=f"wop{m}")
                for c, ntile in enumerate((normt23, normt01)):
                    for hh in range(2):
                        nc.tensor.matmul(
                            po_[:, hh * 512:(hh + 1) * 512],
                            lhsT=ntile[:, m * 128:(m + 1) * 128],
                            rhs=wo_sb[:, c * 1024 + hh * 512:
                                      c * 1024 + hh * 512 + 512],
                            start=(c == 0),
                            stop=(c == 1),
                        )
                wst = psb.tile([128, 1024], F16, tag="wst", bufs=2)
                if cp is None:
                    nc.vector.tensor_copy(wst[:], po_[:])
                else:
                    cp.tensor_copy(wst[:], po_[:])
                nc.sync.dma_start(out[m * 128:(m + 1) * 128, :], wst[:])

            # ---- attention ---------------------------------------------------
            # Each "tile" = one [128,1024] PSUM pair (two atoms), one exp, two
            # F-muls, two AV matmuls.  AV matmuls are deferred by AV_LAG tiles
            # so the PE streams while ACT/DVE work on earlier tiles.
            AV_LAG = 2

            def attend_tile(tb, halves, g, avmms):
                """halves: ((s0, kt0), (s1, kt1)) for sc cols [0:512]/[512:];
                half1's F run sits at a constant positive tile offset from
                half0's, so one fused strided-AP multiply covers both.
                Returns the deferred AV callback."""
                (s0, kt0), (s1, kt1) = halves
                sc = pps.tile([128, 1024], F32, tag="sc", bufs=2)
                for i, (s, kt) in enumerate(halves):
                    po = 64 * (s % 2)
                    nc.tensor.matmul(
                        sc[:, i * 512:(i + 1) * 512],
                        lhsT=kt_sb[tb][po:po + 64, kt * 128:kt * 128 + 128],
                        rhs=qt_sb[tb][po:po + 64, g * 512:(g + 1) * 512],
                    )
                pt = psb.tile([128, 1024], F16, tag="pt", bufs=3)
                nc.scalar.activation(pt[:], sc[:],
                                     mybir.ActivationFunctionType.Exp)
                pm = psb.tile([128, 1024], F16, tag="pm", bufs=3)
                fsb = f23_sb if s0 >= 2 else f01_sb
                fi0 = _f_idx(s0, kt0 - 4 * g)
                fi1 = _f_idx(s1, kt1 - 4 * g)
                delta = fi1 - fi0
                assert delta > 0, (halves, g, fi0, fi1)
                fb = fsb[:]
                fap = bass.AP(
                    fb.tensor, fb.offset + fi0 * 128,
                    [list(fb.ap[0]), [delta * 128, 2], [1, 512]],
                )
                nc.vector.tensor_mul(
                    pm[:].rearrange("p (h c) -> p h c", h=2),
                    pt[:].rearrange("p (h c) -> p h c", h=2),
                    fap)

                def av_mm():
                    for i, (s, kt) in enumerate(halves):
                        av, st, sp = avmms[i]
                        nc.tensor.matmul(
                            av[:],
                            lhsT=vx_sb[kt][:, s * 65:s * 65 + 65],
                            rhs=pm[:, i * 512:(i + 1) * 512], start=st, stop=sp,
                        )
                return av_mm

            wo_pend = []

            def run_pair(pi, tb, s_hi, s_lo, nd_hi_is_full):
                """pi=0: A23 (both slots same band); pi=1: A10 (slot0 full)."""
                for g in range(4):
                    if pi == 0:
                        kts = _unit_kts(SLOT_ND[s_hi], g)
                        tiles = [("pair", kt, kt) for kt in kts]
                    else:
                        r1 = set(_unit_kts(SLOT_ND[1], g))
                        tiles = []
                        solo_run = []
                        for kt in range(NKT):
                            if kt in r1:
                                tiles.append(("pair", kt, kt))
                            else:
                                solo_run.append(kt)
                                if len(solo_run) == 2:
                                    tiles.append(("solo", solo_run[0],
                                                  solo_run[1]))
                                    solo_run = []
                        assert not solo_run
                        tiles.sort(key=lambda t: t[1])
                        if g == 3:
                            # last unit: pairs first so slot-1 finishes (and
                            # its normalize chain starts) while the solo
                            # tiles still stream
                            tiles.sort(key=lambda t: t[0] != "pair")
                    av_hi = pps.tile([65, 512], F32, tag="av", bufs=2,
                                     name=f"av{s_hi}_{g}")
                    av_lo = pps.tile([65, 512], F32, tag="av", bufs=2,
                                     name=f"av{s_lo}_{g}")
                    # start/stop bookkeeping per accumulator
                    n_hi = sum(1 for t in tiles if t[0] == "pair")
                    n_lo = len(tiles) + sum(1 for t in tiles if t[0] == "solo")
                    i_hi = i_lo = 0
                    pend = []
                    for ti, (kind, kt_a, kt_b) in enumerate(tiles):
                        if kind == "pair":
                            mm_lo = (av_lo, i_lo == 0, i_lo == n_lo - 1)
                            i_lo += 1
                            mm_hi = (av_hi, i_hi == 0, i_hi == n_hi - 1)
                            i_hi += 1
                            cb = attend_tile(
                                tb, ((s_lo, kt_a), (s_hi, kt_b)),
                                g, (mm_lo, mm_hi))
                        else:
                            # solo halves ordered (kt_b, kt_a) so the F-run
                            # offset between halves is +1 tile
                            mm_b = (av_lo, i_lo == 0, i_lo == n_lo - 1)
                            i_lo += 1
                            mm_a = (av_lo, i_lo == 0, i_lo == n_lo - 1)
                            i_lo += 1
                            cb = attend_tile(
                                tb, ((s_lo, kt_b), (s_lo, kt_a)),
                                g, (mm_b, mm_a))
                        pend.append(cb)
                        if len(pend) > AV_LAG:
                            pend.pop(0)()
                        if (pi == 1 and g == 3
                                and kind == "pair" and i_hi == n_hi):
                            while pend:
                                pend.pop(0)()
                            normalize(s_hi, g, av_hi, nc.vector)
                        # interleave deferred work at fixed points
                        if pi == 0 and ti == 1 and side:
                            side.pop(0)()
                        if pi == 1 and ti in (2, 5, 8, 11) and wo_pend:
                            # alternate the PSUM->SBUF copy between Scalar and
                            # Vector to balance the two loaded engines
                            emit_wo(wo_pend.pop(0),
                                    cp=(scp if ti == 2 else None))
                    while pend:
                        pend.pop(0)()
                    last = (pi == 1 and g == 3)
                    if last:
                        # hi-slot normalize already emitted mid-unit
                        normalize(s_lo, g, av_lo, nc.vector, keepalive=True)
                    else:
                        normalize(s_hi, g, av_hi)
                        normalize(s_lo, g, av_lo)
                    if pi == 1:
                        wo_pend.extend(range(4 * g, 4 * g + 4))

            run_pair(0, 1, 3, 2, False)
            run_pair(1, 0, 1, 0, True)
            while side:
                side.pop(0)()
            # tail: attention pools are dead — rotate Wo through the sc tag
            # and use the (now idle) Scalar engine for the PSUM->SBUF copies
            for i, m in enumerate(wo_pend):
                emit_wo(m, tag=("sc" if i % 2 else "aux"),
                        cp=(scp if i % 2 else None))
            wo_pend.clear()

    nc.compile()
    return nc


_NC_CACHE = None


def _get_graph():
    global _NC_CACHE
    if _NC_CACHE is None:
        _NC_CACHE = build_graph()
    return _NC_CACHE


def _swizzle_cd(a):
    """[C*128, X] -> [128, C*X] with row p holding chunks c at [c*X:(c+1)*X]."""
    c = a.shape[0] // 128
    return np.ascontiguousarray(
        a.reshape(c, 128, a.shape[1]).transpose(1, 0, 2).reshape(128, -1)
    )


def _slab(a, w):
    """[128, C*S] chunk-major -> [128, (S/w, C, w)] slab-major."""
    c = a.shape[1] // S
    return np.ascontiguousarray(
        a.reshape(128, c, S // w, w).transpose(0, 2, 1, 3).reshape(128, -1)
    )


def _f_block(sl, dd):
    p = np.arange(128)[:, None]
    q = np.arange(128)[None, :]
    return np.exp(-sl * np.abs(dd * 128 + p - q))


def _host_inputs(query, key, value, Wq, Wk, Wv, Wo):
    xqs, xks, xvs = [], [], []
    for b in range(B):
        xqs.append(_slab(_swizzle_cd(query[b].T.astype(np.float32)), 512)
                   .astype(np.float16))
        xks.append(_slab(_swizzle_cd(key[b].T.astype(np.float32)), 512)
                   .astype(np.float16))
        xvs.append(_slab(_swizzle_cd(value[b].T.astype(np.float32)), 128)
                   .astype(np.float16))

    scale = 1.0 / math.sqrt(HD)
    wqs, wks, wvs, f01s, f23s, wos = [], [], [], [], [], []
    for j in range(GROUP):
        cols = np.concatenate(
            [np.arange(64 * h, 64 * h + 64) for h in BINS[j]]
        )
        wqs.append(_swizzle_cd((Wq[:, cols] * scale)
                               .astype(np.float32)).astype(np.float16))
        wks.append(_swizzle_cd(Wk[:, cols].astype(np.float32))
                   .astype(np.float16))
        wvs.append(_swizzle_cd(Wv[:, cols].astype(np.float32)).astype(np.float16))

        f01 = np.zeros((128, NF01 * KT), np.float32)
        f23 = np.zeros((128, NF23 * KT), np.float32)
        for s in range(NSLOT):
            sl = SLOPES[BINS[j][s]]
            dst = f01 if s < 2 else f23
            for dd in range(-SLOT_NDE[s], SLOT_NDE[s] + 1):
                fi = _f_idx(s, dd)
                dst[:, fi * 128:(fi + 1) * 128] = _f_block(sl, dd)
        f01s.append(f01.astype(np.float16))
        f23s.append(f23.astype(np.float16))

        # Wo row shard: chunk 0 rows = (slot2 head, slot3 head) dims,
        # chunk 1 rows = (slot0 head, slot1 head) dims.
        perm = np.concatenate(
            [np.arange(64 * BINS[j][s], 64 * BINS[j][s] + 64)
             for s in (2, 3, 0, 1)]
        )
        w = Wo[perm, :].astype(np.float32)  # [256, 1024]
        wos.append(
            np.concatenate([w[0:128, :], w[128:256, :]], axis=1)
            .astype(np.float16)
        )

    in_maps = []
    for i in range(NCORES):
        b, j = i // GROUP, i % GROUP
        in_maps.append({
            "xq": xqs[b], "xk": xks[b], "xv": xvs[b],
            "wq": wqs[j], "wk": wks[j], "wv": wvs[j], "wo": wos[j],
            "fm01": f01s[j], "fm23": f23s[j],
        })
    return in_maps


def _assemble(results):
    full = np.zeros((B, S, D), np.float32)
    for b in range(B):
        for j in range(GROUP):
            full[b] += results[GROUP * b + j]["out"].astype(np.float32)
    return full


def kernel(**inputs):
    query = np.asarray(inputs["query"], np.float32)
    key = np.asarray(inputs["key"], np.float32)
    value = np.asarray(inputs["value"], np.float32)
    Wq = np.asarray(inputs["Wq"], np.float32)
    Wk = np.asarray(inputs["Wk"], np.float32)
    Wv = np.asarray(inputs["Wv"], np.float32)
    Wo = np.asarray(inputs["Wo"], np.float32)

    nc = _get_graph()
    in_maps = _host_inputs(query, key, value, Wq, Wk, Wv, Wo)
    res = run_bass_kernel_spmd(nc, in_maps, list(range(NCORES)))
    return _assemble(res.results)


# revision 24
# speedup vs baseline: 1.0061x; 1.0061x over previous
"""ALiBi multi-head attention on 8 TRN2 NeuronCores — collective-free.

Sharding: data-parallel over batch (B=2 -> 2 groups of 4 cores), tensor-parallel
over the 16 heads (4 heads per core, Megatron-style column shards of Wq/Wk/Wv).
Each core applies its own ROW shard of Wo to its local normalized attention
outputs and returns a partial [S, 1024] sum; the host adds the four partials
per batch group.  No collectives at all: no CC-queue serialization, no
gather-back DMAs blocking the Sync queue, no cross-core skew on the critical
path, and the Wo matmuls pipeline per q-unit instead of waiting for gathers.

Head assignment is slot-ranked: core group-position j gets heads
{15-j, 11-j, 7-j, 3-j} (slot s holds head BINS[j][s]).  Slots are processed in
parity PAIRS (3,2) then (1,0): the two score matmuls of a pair have K=64 and
live on complementary 64-partition halves (po=64*(s%2)), so the PE runs them
CONCURRENTLY in disjoint row-groups (tile_position auto-derived from
base_partition) — near-2x score throughput.  Each pair writes one [128,1024]
PSUM tile and gets a single pair-batched exp on the Scalar engine (the
352-cycle per-ACTIVATE overhead is paid once per two atoms).  Slot-0 atoms
outside slot-1's band are batched two-per-exp the same way.

The ALiBi factor tiles F = exp(-slope*|k-q|) (fp16, Toeplitz: one tile per
tile-diagonal offset) carry each core's actual slopes and double as the band
mask (F underflows to 0 outside a head's band).  Softmax skips the
max-subtraction (scores ~ N(0, 0.45^2), fp32 exp cannot overflow).  V carries
a fused ones-column so the attention-value matmul also emits the softmax
denominators (row 64) for free.

Engine placement: exp on Scalar; F-multiplies + PSUM->SBUF copies on Vector;
the normalize multiplies (SBUF-only) on GpSimd (which has no other work now);
projection copies on Scalar while it is otherwise idle (pre-attention).  K/Q
projections run stationary-weight c-loops (2 q-blocks share each LDWEIGHTS).
"""

import math
import sys

import numpy as np
import ml_dtypes

F8NP = ml_dtypes.float8_e4m3fn

if "/opt/trn_rl_repo" not in sys.path:
    sys.path.insert(0, "/opt/trn_rl_repo")

import concourse.bass as bass  # noqa: E402
import concourse.mybir as mybir  # noqa: E402
import concourse.tile as tile  # noqa: E402
from concourse import bacc  # noqa: E402
from concourse.bass_utils import run_bass_kernel_spmd  # noqa: E402

B, S, D, H, HD = 2, 2048, 1024, 16, 64
NCORES = 8
GROUP = 4          # cores per batch group
NSLOT = 4          # head slots per core
KT = 128           # k (key position) tile size
NKT = S // KT      # 16
NDC = D // 128     # 8 contraction chunks
T_FACTOR = 8.0     # band radius = T_FACTOR / slope (exp(-8) tail ~ 2e-4 bias)

F16 = mybir.dt.float16
F32 = mybir.dt.float32
F8 = mybir.dt.float8e4
W8SCALE = 64.0  # fp8 weight pre-scale (values land in e4m3 normal range)

SLOPES = [2.0 ** (-0.5 * (h + 1)) for h in range(H)]
# BINS[j][s] = head of slot s on cores j and j+4
BINS = [[15 - j, 11 - j, 7 - j, 3 - j] for j in range(GROUP)]
# slot pairs processed (hi, lo): hi at partitions 64:128, lo at 0:64
PAIRS = [(3, 2), (1, 0)]


def _slot_nd():
    """Max |k_tile - q_tile| included per slot (widest head in the slot)."""
    nds = []
    for s in range(NSLOT):
        t = max(
            min(S - 1, int(math.ceil(T_FACTOR / SLOPES[BINS[j][s]])))
            for j in range(GROUP)
        )
        nds.append(min(NKT - 1, (t + KT - 1) // KT))
    return nds


SLOT_ND = _slot_nd()  # [15, 4, 1, 1]
# F tiles must cover offsets (kt - q_tile) for every q_tile in a 4-tile group:
# extend the band by 3 (values there underflow to 0 in fp16 -> free masking).
SLOT_NDE = [min(NKT - 1, nd + (7 if nd >= 8 else 3)) for nd in SLOT_ND]
# per-slot F bases, LOCAL to the fm tensor the slot lives in:
# fm01 holds slots 0,1; fm23 holds slots 2,3.
F_BASE = {}
_acc = 0
for _s in (0, 1):
    F_BASE[_s] = _acc
    _acc += 2 * SLOT_NDE[_s] + 1
NF01 = _acc
_acc = 0
for _s in (2, 3):
    F_BASE[_s] = _acc
    _acc += 2 * SLOT_NDE[_s] + 1
NF23 = _acc


def _f_idx(s, dd):
    """Index of the F tile for slot s, diagonal offset dd = k_tile - q_tile."""
    return F_BASE[s] + SLOT_NDE[s] - dd


def _unit_kts(nd, g):
    return list(range(max(0, 4 * g - nd), min(NKT - 1, 4 * g + 3 + nd) + 1))


def build_graph():
    nc = bacc.Bacc("TRN2", target_bir_lowering=False, debug=False,
                   num_devices=NCORES)
    # xv in k-tile slabs [128, (slab, c, 512)]; xq/xk in q-block slabs.
    xq = nc.dram_tensor("xq", [128, NDC * S], F16, kind="ExternalInput")
    xk = nc.dram_tensor("xk", [128, NDC * S], F16, kind="ExternalInput")
    xv = nc.dram_tensor("xv", [128, NDC * S], F16, kind="ExternalInput")
    wq = nc.dram_tensor("wq", [128, NDC * 256], F16, kind="ExternalInput")
    wk = nc.dram_tensor("wk", [128, NDC * 256], F16, kind="ExternalInput")
    wv = nc.dram_tensor("wv", [128, NDC * 256], F16, kind="ExternalInput")
    # wo: two rhs chunks [128, 1024]: chunk 0 = (slot2, slot3) head rows,
    # chunk 1 = (slot0, slot1) head rows (matching normt partition layout).
    wo = nc.dram_tensor("wo", [128, 2 * 1024], F16, kind="ExternalInput")
    fm01 = nc.dram_tensor("fm01", [128, NF01 * KT], F16, kind="ExternalInput")
    fm23 = nc.dram_tensor("fm23", [128, NF23 * KT], F16, kind="ExternalInput")
    out = nc.dram_tensor("out", [S, 1024], F16, kind="ExternalOutput")

    with tile.TileContext(nc) as tc:
        with (
            tc.tile_pool(name="wpool", bufs=1) as wpool,
            tc.tile_pool(name="persist", bufs=1) as persist,
            tc.tile_pool(name="xpool", bufs=1) as xpool,
            tc.tile_pool(name="dram", bufs=1, space="DRAM") as dram,
            tc.tile_pool(name="pps", bufs=1, space="PSUM") as pps,
            tc.tile_pool(name="psb", bufs=1) as psb,
            tc.tile_pool(name="nsb", bufs=1) as nsb,
        ):
            wv_sb = wpool.tile([128, NDC * 256], F16)
            wq_sb = wpool.tile([128, NDC * 256], F16)
            wk_sb = wpool.tile([128, NDC * 256], F16)
            wo_sb = wpool.tile([128, 2 * 1024], F16)
            f01_sb = wpool.tile([128, NF01 * KT], F16)
            f23_sb = wpool.tile([128, NF23 * KT], F16)

            xv_sl = [xpool.tile([128, NDC * 512], F16, tag="xv", bufs=4,
                                name=f"xv{n}") for n in range(4)]
            xk_sl = [xpool.tile([128, NDC * 512], F16, tag="xkq", bufs=8,
                                name=f"xk{n}") for n in range(4)]
            xq_sl = [xpool.tile([128, NDC * 512], F16, tag="xkq", bufs=8,
                                name=f"xq{n}") for n in range(4)]

            # Q^T/K^T: pair tile t holds slot 2t (partitions 0:64) and slot
            # 2t+1 (partitions 64:128).  V natural layout per k-tile with a
            # ones column per (k_tile, slot) at vx[kt][:, s*65 + 64].
            qt_sb = [persist.tile([128, S], F16, name=f"qt{m}") for m in range(2)]
            kt_sb = [persist.tile([128, S], F16, name=f"kt{m}") for m in range(2)]
            vx_sb = [persist.tile([128, NSLOT * 65], F16, name=f"vx{k}")
                     for k in range(NKT)]
            for k in range(NKT):
                nc.vector.memset(
                    vx_sb[k][:].rearrange("p (s e) -> p s e", s=NSLOT)[:, :, 64:65],
                    1.0,
                )
            # normalized attention outputs: chunk (s2|s3) and (s0|s1),
            # slot s at partitions 64*(s%2).
            normt23 = persist.tile([128, S], F16, name="n23")
            normt01 = persist.tile([128, S], F16, name="n01")
            NTILE = {0: normt01, 1: normt01, 2: normt23, 3: normt23}

            # ---- input DMAs (consumption order) ------------------------------
            def ldx(sl, t, n):
                nc.sync.dma_start(sl[n][:],
                                  t[:, n * NDC * 512:(n + 1) * NDC * 512])

            nc.sync.dma_start(wv_sb[:], wv[:])
            ldx(xv_sl, xv, 0)
            ldx(xv_sl, xv, 1)
            nc.sync.dma_start(wk_sb[:], wk[:])
            nc.sync.dma_start(wq_sb[:], wq[:])
            ldx(xk_sl, xk, 0)
            ldx(xq_sl, xq, 0)
            ldx(xv_sl, xv, 2)
            ldx(xk_sl, xk, 1)
            ldx(xq_sl, xq, 1)
            ldx(xv_sl, xv, 3)
            nc.sync.dma_start(f23_sb[:], fm23[:])
            for n in range(2, 4):
                ldx(xk_sl, xk, n)
                ldx(xq_sl, xq, n)
            nc.sync.dma_start(f01_sb[:], fm01[:])
            nc.sync.dma_start(wo_sb[:], wo[:])

            # preload the exp table set while the PE does projections
            warm = nsb.tile([1, 16], F16, tag="warm")
            nc.scalar.activation(warm[:], vx_sb[0][0:1, 0:16],
                                 mybir.ActivationFunctionType.Exp)

            # ---- projections -------------------------------------------------
            # V (natural layout, fused ones column untouched at col 64).
            # xv slabs are kt-major / c-inner so kt 0 only needs the first
            # quarter of slab 0 (earlier start while the DMA engine ramps).
            for kt in range(NKT):
                ps = pps.tile([128, 1024], F32, tag="sc", bufs=2,
                              name=f"vps{kt}")
                base = (kt % 4) * NDC * 128
                for c in range(NDC):
                    nc.tensor.matmul(
                        ps[:, 0:256],
                        lhsT=xv_sl[kt // 4][:, base + c * 128:
                                            base + c * 128 + 128],
                        rhs=wv_sb[:, c * 256:(c + 1) * 256],
                        start=(c == 0),
                        stop=(c == NDC - 1),
                    )
                nc.scalar.copy(
                    vx_sb[kt][:].rearrange("p (s e) -> p s e", s=NSLOT)[:, :, 0:64],
                    ps[:, 0:256].rearrange("p (s e) -> p s e", e=64),
                )

            # K^T/Q^T: stationary-weight c-loop over two q-blocks at a time
            # (the two MMs of each c share one LDWEIGHTS' worth of re-load).
            def qk_sub(wsb, xsl, dst, mb, nbs, eng):
                ps = pps.tile([128, 1024], F32, tag="aux", bufs=1,
                              name=f"qk{mb}{nbs[0]}")
                for c in range(NDC):
                    for i, nb in enumerate(nbs):
                        nc.tensor.matmul(
                            ps[:, i * 512:(i + 1) * 512],
                            lhsT=wsb[:, c * 256 + mb * 128:
                                     c * 256 + mb * 128 + 128],
                            rhs=xsl[nb][:, c * 512:(c + 1) * 512],
                            start=(c == 0),
                            stop=(c == NDC - 1),
                        )
                for i, nb in enumerate(nbs):
                    eng(dst[:, nb * 512:(nb + 1) * 512],
                        ps[:, i * 512:(i + 1) * 512])

            class _ScalarCopy:
                def tensor_copy(self, dst, src):
                    nc.scalar.copy(dst, src)

            scp = _ScalarCopy()

            def sc_cp(dst, src):
                nc.scalar.copy(dst, src)

            def ve_cp(dst, src):
                nc.vector.tensor_copy(dst, src)

            qk_sub(wk_sb, xk_sl, kt_sb[1], 1, (0, 1), sc_cp)
            qk_sub(wq_sb, xq_sl, qt_sb[1], 1, (0, 1), sc_cp)
            qk_sub(wk_sb, xk_sl, kt_sb[1], 1, (2, 3), sc_cp)
            qk_sub(wq_sb, xq_sl, qt_sb[1], 1, (2, 3), sc_cp)
            side = [
                lambda: qk_sub(wk_sb, xk_sl, kt_sb[0], 0, (0, 1), ve_cp),
                lambda: qk_sub(wk_sb, xk_sl, kt_sb[0], 0, (2, 3), ve_cp),
                lambda: qk_sub(wq_sb, xq_sl, qt_sb[0], 0, (0, 1), ve_cp),
                lambda: qk_sub(wq_sb, xq_sl, qt_sb[0], 0, (2, 3), ve_cp),
            ]

            # ---- normalize ---------------------------------------------------
            def keep_pe_warm(lhsT, rhs):
                """Junk matmuls gated on a normalize-chain artifact: keep the
                PE's HAM activity window alive through the final normalize
                latency so the tail Wo matmuls run at full clock."""
                dps = pps.tile([128, 1024], F32, tag="sc", bufs=2, name="dps")
                for r in range(2):
                    nc.tensor.matmul(
                        dps[0:lhsT.free_size(), 0:rhs.shape[-1]],
                        lhsT=lhsT, rhs=rhs, start=True, stop=True,
                    )

            def normalize(s, g, av, mul_eng=None, keepalive=False):
                av_sb = nsb.tile([65, 512], F32, tag="avs", bufs=4)
                nc.vector.tensor_copy(av_sb[:], av[:])
                dsh = nsb.tile([32, 16], F32, tag="dsh", bufs=4)
                nc.sync.dma_start(dsh[:], av_sb[64:65, :])
                rec = nsb.tile([32, 16], F32, tag="rec", bufs=4)
                nc.vector.reciprocal(rec[:], dsh[:])
                rec16 = nsb.tile([32, 16], F16, tag="rec16", bufs=4)
                nc.vector.tensor_copy(rec16[:], rec[:])
                rdr = dram.tile([1, 512], F16, name=f"rdr{s}_{g}",
                                bufs=4, tag="rdr")
                nc.sync.dma_start(rdr[:], rec16[:])
                if keepalive:
                    keep_pe_warm(lhsT=rec16[:], rhs=qt_sb[0][0:32, 0:512])
                bc_sb = nsb.tile([64, 512], F16, tag="bcs", bufs=4)
                nc.sync.dma_start(
                    bc_sb[:],
                    rdr[:].partition_broadcast(64).squeeze(1),
                )
                if keepalive:
                    keep_pe_warm(lhsT=kt_sb[0][0:64, 0:128], rhs=bc_sb[:])
                po = 64 * (s % 2)
                (mul_eng or nc.gpsimd).tensor_mul(
                    NTILE[s][po:po + 64, g * 512:(g + 1) * 512],
                    av_sb[0:64, :],
                    bc_sb[:],
                )

            # ---- Wo (local row shard, per q-tile) ----------------------------
            def emit_wo(m, tag="aux", cp=None):
                po_ = pps.tile([128, 1024], F32, tag=tag, bufs=2 if tag == "sc" else 1,
                               name=f"wop{m}")
                for c, ntile in enumerate((normt23, normt01)):
                    for hh in range(2):
                        nc.tensor.matmul(
                            po_[:, hh * 512:(hh + 1) * 512],
                            lhsT=ntile[:, m * 128:(m + 1) * 128],
                            rhs=wo_sb[:, c * 1024 + hh * 512:
                                      c * 1024 + hh * 512 + 512],
                            start=(c == 0),
                            stop=(c == 1),
                        )
                wst = psb.tile([128, 1024], F16, tag="wst", bufs=2)
                if cp is None:
                    nc.vector.tensor_copy(wst[:], po_[:])
                else:
                    cp.tensor_copy(wst[:], po_[:])
                nc.sync.dma_start(out[m * 128:(m + 1) * 128, :], wst[:])

            # ---- attention ---------------------------------------------------
            # Each "tile" = one [128,1024] PSUM pair (two atoms), one exp, two
            # F-muls, two AV matmuls.  AV matmuls are deferred by AV_LAG tiles
            # so the PE streams while ACT/DVE work on earlier tiles.
            AV_LAG = 2

            def attend_tile(tb, halves, g, avmms):
                """halves: ((s0, kt0), (s1, kt1)) for sc cols [0:512]/[512:];
                half1's F run sits at a constant positive tile offset from
                half0's, so one fused strided-AP multiply covers both.
                Returns the deferred AV callback."""
                (s0, kt0), (s1, kt1) = halves
                sc = pps.tile([128, 1024], F32, tag="sc", bufs=2)
                for i, (s, kt) in enumerate(halves):
                    po = 64 * (s % 2)
                    nc.tensor.matmul(
                        sc[:, i * 512:(i + 1) * 512],
                        lhsT=kt_sb[tb][po:po + 64, kt * 128:kt * 128 + 128],
                        rhs=qt_sb[tb][po:po + 64, g * 512:(g + 1) * 512],
                    )
                pt = psb.tile([128, 1024], F16, tag="pt", bufs=3)
                nc.scalar.activation(pt[:], sc[:],
                                     mybir.ActivationFunctionType.Exp)
                pm = psb.tile([128, 1024], F16, tag="pm", bufs=3)
                fsb = f23_sb if s0 >= 2 else f01_sb
                fi0 = _f_idx(s0, kt0 - 4 * g)
                fi1 = _f_idx(s1, kt1 - 4 * g)
                delta = fi1 - fi0
                assert delta > 0, (halves, g, fi0, fi1)
                fb = fsb[:]
                fap = bass.AP(
                    fb.tensor, fb.offset + fi0 * 128,
                    [list(fb.ap[0]), [delta * 128, 2], [1, 512]],
                )
                nc.vector.tensor_mul(
                    pm[:].rearrange("p (h c) -> p h c", h=2),
                    pt[:].rearrange("p (h c) -> p h c", h=2),
                    fap)

                def av_mm():
                    for i, (s, kt) in enumerate(halves):
                        av, st, sp = avmms[i]
                        nc.tensor.matmul(
                            av[:],
                            lhsT=vx_sb[kt][:, s * 65:s * 65 + 65],
                            rhs=pm[:, i * 512:(i + 1) * 512], start=st, stop=sp,
                        )
                return av_mm

            wo_pend = []

            def run_pair(pi, tb, s_hi, s_lo, nd_hi_is_full):
                """pi=0: A23 (both slots same band); pi=1: A10 (slot0 full)."""
                for g in range(4):
                    if pi == 0:
                        kts = _unit_kts(SLOT_ND[s_hi], g)
                        tiles = [("pair", kt, kt) for kt in kts]
                    else:
                        r1 = set(_unit_kts(SLOT_ND[1], g))
                        tiles = []
                        solo_run = []
                        for kt in range(NKT):
                            if kt in r1:
                                tiles.append(("pair", kt, kt))
                            else:
                                solo_run.append(kt)
                                if len(solo_run) == 2:
                                    tiles.append(("solo", solo_run[0],
                                                  solo_run[1]))
                                    solo_run = []
                        assert not solo_run
                        tiles.sort(key=lambda t: t[1])
                        if g == 3:
                            # last unit: pairs first so slot-1 finishes (and
                            # its normalize chain starts) while the solo
                            # tiles still stream
                            tiles.sort(key=lambda t: t[0] != "pair")
                    av_hi = pps.tile([65, 512], F32, tag="av", bufs=2,
                                     name=f"av{s_hi}_{g}")
                    av_lo = pps.tile([65, 512], F32, tag="av", bufs=2,
                                     name=f"av{s_lo}_{g}")
                    # start/stop bookkeeping per accumulator
                    n_hi = sum(1 for t in tiles if t[0] == "pair")
                    n_lo = len(tiles) + sum(1 for t in tiles if t[0] == "solo")
                    i_hi = i_lo = 0
                    pend = []
                    for ti, (kind, kt_a, kt_b) in enumerate(tiles):
                        if kind == "pair":
                            mm_lo = (av_lo, i_lo == 0, i_lo == n_lo - 1)
                            i_lo += 1
                            mm_hi = (av_hi, i_hi == 0, i_hi == n_hi - 1)
                            i_hi += 1
                            cb = attend_tile(
                                tb, ((s_lo, kt_a), (s_hi, kt_b)),
                                g, (mm_lo, mm_hi))
                        else:
                            # solo halves ordered (kt_b, kt_a) so the F-run
                            # offset between halves is +1 tile
                            mm_b = (av_lo, i_lo == 0, i_lo == n_lo - 1)
                            i_lo += 1
                            mm_a = (av_lo, i_lo == 0, i_lo == n_lo - 1)
                            i_lo += 1
                            cb = attend_tile(
                                tb, ((s_lo, kt_b), (s_lo, kt_a)),
                                g, (mm_b, mm_a))
                        pend.append(cb)
                        if len(pend) > AV_LAG:
                            pend.pop(0)()
                        if (pi == 1 and g == 3
                                and kind == "pair" and i_hi == n_hi):
                            while pend:
                                pend.pop(0)()
                            normalize(s_hi, g, av_hi, nc.vector)
                        # interleave deferred work at fixed points
                        if pi == 0 and ti == 1 and side:
                            side.pop(0)()
                        if pi == 1 and ti in (2, 5, 8, 11) and wo_pend:
                            # alternate the PSUM->SBUF copy between Scalar and
                            # Vector to balance the two loaded engines
                            emit_wo(wo_pend.pop(0),
                                    cp=(scp if ti == 2 else None))
                    while pend:
                        pend.pop(0)()
                    last = (pi == 1 and g == 3)
                    if last:
                        # hi-slot normalize already emitted mid-unit
                        normalize(s_lo, g, av_lo, nc.vector, keepalive=True)
                    else:
                        normalize(s_hi, g, av_hi)
                        normalize(s_lo, g, av_lo)
                    if pi == 1:
                        wo_pend.extend(range(4 * g, 4 * g + 4))

            run_pair(0, 1, 3, 2, False)
            run_pair(1, 0, 1, 0, True)
            while side:
                side.pop(0)()
            # tail: attention pools are dead — rotate Wo through the sc tag
            # and use the (now idle) Scalar engine for the PSUM->SBUF copies
            for i, m in enumerate(wo_pend):
                emit_wo(m, tag=("sc" if i % 2 else "aux"),
                        cp=(scp if i % 2 else None))
            wo_pend.clear()

    nc.compile()
    return nc


_NC_CACHE = None


def _get_graph():
    global _NC_CACHE
    if _NC_CACHE is None:
        _NC_CACHE = build_graph()
    return _NC_CACHE


def _swizzle_cd(a):
    """[C*128, X] -> [128, C*X] with row p holding chunks c at [c*X:(c+1)*X]."""
    c = a.shape[0] // 128
    return np.ascontiguousarray(
        a.reshape(c, 128, a.shape[1]).transpose(1, 0, 2).reshape(128, -1)
    )


def _slab(a, w):
    """[128, C*S] chunk-major -> [128, (S/w, C, w)] slab-major."""
    c = a.shape[1] // S
    return np.ascontiguousarray(
        a.reshape(128, c, S // w, w).transpose(0, 2, 1, 3).reshape(128, -1)
    )


def _f_block(sl, dd):
    p = np.arange(128)[:, None]
    q = np.arange(128)[None, :]
    return np.exp(-sl * np.abs(dd * 128 + p - q))


def _host_inputs(query, key, value, Wq, Wk, Wv, Wo):
    xqs, xks, xvs = [], [], []
    for b in range(B):
        xqs.append(_slab(_swizzle_cd(query[b].T.astype(np.float32)), 512)
                   .astype(np.float16))
        xks.append(_slab(_swizzle_cd(key[b].T.astype(np.float32)), 512)
                   .astype(np.float16))
        xvs.append(_slab(_swizzle_cd(value[b].T.astype(np.float32)), 128)
                   .astype(np.float16))

    scale = 1.0 / math.sqrt(HD)
    wqs, wks, wvs, f01s, f23s, wos = [], [], [], [], [], []
    for j in range(GROUP):
        cols = np.concatenate(
            [np.arange(64 * h, 64 * h + 64) for h in BINS[j]]
        )
        wqs.append(_swizzle_cd((Wq[:, cols] * scale)
                               .astype(np.float32)).astype(np.float16))
        wks.append(_swizzle_cd(Wk[:, cols].astype(np.float32))
                   .astype(np.float16))
        wvs.append(_swizzle_cd(Wv[:, cols].astype(np.float32)).astype(np.float16))

        f01 = np.zeros((128, NF01 * KT), np.float32)
        f23 = np.zeros((128, NF23 * KT), np.float32)
        for s in range(NSLOT):
            sl = SLOPES[BINS[j][s]]
            dst = f01 if s < 2 else f23
            for dd in range(-SLOT_NDE[s], SLOT_NDE[s] + 1):
                fi = _f_idx(s, dd)
                dst[:, fi * 128:(fi + 1) * 128] = _f_block(sl, dd)
        f01s.append(f01.astype(np.float16))
        f23s.append(f23.astype(np.float16))

        # Wo row shard: chunk 0 rows = (slot2 head, slot3 head) dims,
        # chunk 1 rows = (slot0 head, slot1 head) dims.
        perm = np.concatenate(
            [np.arange(64 * BINS[j][s], 64 * BINS[j][s] + 64)
             for s in (2, 3, 0, 1)]
        )
        w = Wo[perm, :].astype(np.float32)  # [256, 1024]
        wos.append(
            np.concatenate([w[0:128, :], w[128:256, :]], axis=1)
            .astype(np.float16)
        )

    in_maps = []
    for i in range(NCORES):
        b, j = i // GROUP, i % GROUP
        in_maps.append({
            "xq": xqs[b], "xk": xks[b], "xv": xvs[b],
            "wq": wqs[j], "wk": wks[j], "wv": wvs[j], "wo": wos[j],
            "fm01": f01s[j], "fm23": f23s[j],
        })
    return in_maps


def _assemble(results):
    full = np.zeros((B, S, D), np.float32)
    for b in range(B):
        for j in range(GROUP):
            full[b] += results[GROUP * b + j]["out"].astype(np.float32)
    return full


def kernel(**inputs):
    query = np.asarray(inputs["query"], np.float32)
    key = np.asarray(inputs["key"], np.float32)
    value = np.asarray(inputs["value"], np.float32)
    Wq = np.asarray(inputs["Wq"], np.float32)
    Wk = np.asarray(inputs["Wk"], np.float32)
    Wv = np.asarray(inputs["Wv"], np.float32)
    Wo = np.asarray(inputs["Wo"], np.float32)

    nc = _get_graph()
    in_maps = _host_inputs(query, key, value, Wq, Wk, Wv, Wo)
    res = run_bass_kernel_spmd(nc, in_maps, list(range(NCORES)))
    return _assemble(res.results)
